# revision 1
# baseline (speedup 1.0000x reference)
"""DOM pooling (segment mean+max over pulses, then linear projection) on 8 trn2 cores.

Strategy:
  Host: bucket DOMs by exact pulse count k ("classes"); deal DOMs of each class
  round-robin across the 8 cores so every core has an identical class structure
  (same per-class DOM count m_k, padded with dummy DOMs). Full 128-DOM windows
  are emitted per class; the leftover (<128) DOMs of every class are packed
  together into shared "ragged" windows (sorted by k, per-DOM slots padded to
  the window capacity by duplicating the DOM's first slot — max-neutral; the
  sum is corrected on device by subtracting padcnt*x0 and scaled by 1/k).
  Each core gets a gathered slot buffer where a DOM's k pulse embeddings are
  stored embed-major (64 x k contiguous), so device reads are sequential.

  Device (one NEFF, SPMD on 8 cores), per 128-DOM window:
    - batched contiguous DMA loads
    - DVE reduce_sum / reduce_max over the slots (contiguous inner axis)
    - PE transpose of [sum|max] concat (128x128) -> PSUM (feat-major)
    - projection matmul out^T = Wk^T.T @ concatT (mean scaling 1/k folded into
      the sum-feature rows of the per-class weights; ragged windows scale on
      DVE and use unscaled weights)
    - ACT adds bias during PSUM->SBUF copy; batched DMA writes out^T.

  Host: scatter per-core transposed outputs back to the full (num_doms, 64).
"""
import sys

import numpy as np

for _p in ("/opt/trn_rl_repo",):
    if _p not in sys.path:
        sys.path.append(_p)

from concourse import bacc
import concourse.mybir as mybir
import concourse.tile as tile
from concourse.bass_utils import run_bass_kernel_spmd
from concourse.masks import make_identity

NCORES = 8
D = 64
FP32 = mybir.dt.float32

last_exec_ns = None  # set when KERNEL_TRACE=1


def _plan(counts):
    """Class/window structure shared by all cores (derived from global counts).

    Returns:
      full_cls: list of (k, fw, col0, base) classes with fw full windows
      rag_cls:  list of (k, r) leftover doms per class (class order)
      rag_win:  list of (k_w, base, col0) ragged windows
      ndcp:     output columns per core
      s_elems:  slot buffer elements per core
    """
    kmax = int(counts.max()) if counts.size else 0
    n_k = np.bincount(counts, minlength=kmax + 1)
    full_cls = []
    rag_cls = []
    col = 0
    slot = 0
    for k in range(1, kmax + 1):
        if n_k[k] == 0:
            continue
        m = -(-int(n_k[k]) // NCORES)
        fw = m // 128
        r = m % 128
        if fw:
            full_cls.append((k, fw, col, slot))
            col += fw * 128
            slot += fw * 128 * k * D
        if r:
            rag_cls.append((k, r))
    # ragged windows: doms in class order (ascending k); capacity = max k in win
    rag_win = []
    R = sum(r for _, r in rag_cls)
    if R:
        ks = np.concatenate([np.full(r, k, np.int32) for k, r in rag_cls])
        RW = -(-R // 128)
        for j in range(RW):
            seg = ks[j * 128 : (j + 1) * 128]
            kw = int(seg.max())
            rag_win.append((kw, slot, col + j * 128))
            slot += 128 * kw * D
        col += RW * 128
    return full_cls, rag_cls, rag_win, col, slot


def _build_nc(full_cls, rag_win, ndcp, s_elems, nwt, nrw):
    nc = bacc.Bacc(None)
    slots_t = nc.dram_tensor("slots", [s_elems], FP32, kind="ExternalInput")
    wts_t = nc.dram_tensor("wts", [nwt * 128, D], FP32, kind="ExternalInput")
    b_t = nc.dram_tensor("b", [D, 1], FP32, kind="ExternalInput")
    if nrw:
        rt_t = nc.dram_tensor("rt", [nrw * 128, 2], FP32, kind="ExternalInput")
    out_t = nc.dram_tensor("out", [D, ndcp], FP32, kind="ExternalOutput")

    with tile.TileContext(nc) as tc:
        with (
            tc.tile_pool(name="const", bufs=1) as constp,
            tc.tile_pool(name="inp", bufs=6) as inp,
            tc.tile_pool(name="mid", bufs=4) as midp,
            tc.tile_pool(name="outp", bufs=4) as outp,
            tc.tile_pool(name="psA", bufs=4, space="PSUM") as psA,
            tc.tile_pool(name="psB", bufs=4, space="PSUM") as psB,
        ):
            ident = constp.tile([128, 128], FP32)
            make_identity(nc, ident[:])
            wt_sb = constp.tile([128, nwt * D], FP32)
            nc.sync.dma_start(
                wt_sb[:].rearrange("p (j e) -> p j e", e=D),
                wts_t[:, :].rearrange("(j p) e -> p j e", p=128),
            )
            b_sb = constp.tile([D, 1], FP32)
            nc.sync.dma_start(b_sb[:], b_t[:])
            if nrw:
                rt_sb = constp.tile([128, nrw * 2], FP32)
                nc.sync.dma_start(
                    rt_sb[:].rearrange("p (j c) -> p j c", c=2),
                    rt_t[:, :].rearrange("(j p) c -> p j c", p=128),
                )

            def window_tail(cat, out_ap, p, jwt):
                """cat: (p, 128) [sum|max] slice; out_ap: (D, p) slice."""
                catT_ps = psA.tile([128, 128], FP32, space="PSUM", tag="ps")
                nc.tensor.transpose(
                    out=catT_ps[:, :p], in_=cat, identity=ident[:p, :p]
                )
                catT = midp.tile([128, 128], FP32, tag="catT")
                nc.scalar.copy(catT[:, :p], catT_ps[:, :p])
                proj_ps = psB.tile([D, 128], FP32, space="PSUM", tag="proj")
                nc.tensor.matmul(
                    proj_ps[:, :p],
                    lhsT=wt_sb[:, jwt * D : (jwt + 1) * D],
                    rhs=catT[:, :p],
                    start=True,
                    stop=True,
                )
                nc.scalar.activation(
                    out_ap, proj_ps[:, :p],
                    mybir.ActivationFunctionType.Identity, bias=b_sb[:, :1],
                )

            # ---- full per-class windows --------------------------------
            for jcls, (k, fw, col0, base) in enumerate(full_cls):
                G = max(1, min(8, 64 // k))
                g = 0
                while g < fw:
                    gw = min(G, fw - g)
                    in_t = inp.tile([128, gw * k * D], FP32, tag="in")
                    src = slots_t[
                        base + g * 128 * k * D : base + (g + gw) * 128 * k * D
                    ].rearrange("(w d f) -> d w f", w=gw, d=128)
                    nc.sync.dma_start(
                        in_t[:].rearrange("d (w f) -> d w f", w=gw), src
                    )
                    cat_g = midp.tile([128, 8 * 128], FP32, tag="cat")
                    co = cat_g[:, : gw * 128].rearrange("d (w c) -> d w c", c=128)
                    if k == 1:
                        v = in_t[:].rearrange("d (w e) -> d w e", w=gw)
                        nc.vector.tensor_copy(co[:, :, 0:D], v)
                        nc.vector.tensor_copy(co[:, :, D:128], v)
                    else:
                        view = in_t[:].rearrange("d (w e s) -> d w e s", w=gw, s=k)
                        nc.vector.reduce_sum(co[:, :, 0:D], view, axis=mybir.AxisListType.X)
                        nc.vector.reduce_max(co[:, :, D:128], view, axis=mybir.AxisListType.X)
                    out_sb = outp.tile([64, 8 * 128], FP32, tag="out")
                    for w in range(gw):
                        window_tail(
                            cat_g[:, w * 128 : (w + 1) * 128],
                            out_sb[:, w * 128 : (w + 1) * 128],
                            128, jcls,
                        )
                    nc.sync.dma_start(
                        out_t[:, col0 + g * 128 : col0 + (g + gw) * 128],
                        out_sb[:, : gw * 128],
                    )
                    g += gw

            # ---- ragged windows (mixed k, capacity k_w) ----------------
            juns = len(full_cls)  # unscaled weight block index
            for j, (kw, base, col0) in enumerate(rag_win):
                in_t = inp.tile([128, kw * D], FP32, tag="in")
                nc.sync.dma_start(
                    in_t[:], slots_t[base : base + 128 * kw * D].rearrange(
                        "(d f) -> d f", d=128
                    ),
                )
                cat_g = midp.tile([128, 8 * 128], FP32, tag="cat")
                view = in_t[:].rearrange("d (e s) -> d e s", s=kw)
                sraw = midp.tile([128, D], FP32, tag="sraw")
                nc.vector.reduce_sum(sraw[:], view, axis=mybir.AxisListType.X)
                nc.vector.reduce_max(cat_g[:, D:128], view, axis=mybir.AxisListType.X)
                # sum correction: (sraw - padcnt*x0) * recip_k
                x0 = view[:, :, 0]
                tmp = midp.tile([128, D], FP32, tag="tmp")
                nc.vector.tensor_scalar_mul(
                    tmp[:], x0, rt_sb[:, j * 2 + 1 : j * 2 + 2]
                )
                nc.vector.tensor_tensor(
                    out=tmp[:], in0=sraw[:], in1=tmp[:], op=mybir.AluOpType.subtract
                )
                nc.vector.tensor_scalar_mul(
                    cat_g[:, 0:D], tmp[:], rt_sb[:, j * 2 : j * 2 + 1]
                )
                out_sb = outp.tile([64, 8 * 128], FP32, tag="out")
                window_tail(cat_g[:, 0:128], out_sb[:, :128], 128, juns)
                nc.sync.dma_start(out_t[:, col0 : col0 + 128], out_sb[:, :128])
    nc.finalize()
    return nc


def kernel(pulse_embeddings, pulse_to_dom_idx, num_doms, proj_w, proj_b):
    global last_exec_ns
    import os

    E = np.ascontiguousarray(np.asarray(pulse_embeddings, dtype=np.float32))
    idx = np.asarray(pulse_to_dom_idx).astype(np.int64)
    nd = int(num_doms)
    W = np.asarray(proj_w, dtype=np.float32)   # (D, 2D)
    b = np.asarray(proj_b, dtype=np.float32)   # (D,)

    counts = np.bincount(idx, minlength=nd)
    full_cls, rag_cls, rag_win, ndcp, s_elems = _plan(counts)
    nwt = len(full_cls) + 1
    nrw = len(rag_win)

    # ---- host-side dom assignment --------------------------------------
    dom_order = np.argsort(counts, kind="stable")
    cs = counts[dom_order]
    n0 = int((counts == 0).sum())
    dom_core = np.full(nd, -1, np.int32)
    dom_col = np.full(nd, -1, np.int32)

    # per-class bookkeeping (shared across cores)
    kmax = int(counts.max()) if counts.size else 0
    n_k = np.bincount(counts, minlength=kmax + 1)
    full_map = {k: (jc, fw, col0, base) for jc, (k, fw, col0, base) in enumerate(full_cls)}
    # ragged: position of each class's leftover run inside the ragged region
    rag_off = {}
    ro = 0
    for k, r in rag_cls:
        rag_off[k] = ro
        ro += r
    R = ro
    rag_col0 = rag_win[0][2] - 0 if rag_win else ndcp  # col of ragged dom 0
    if rag_win:
        rag_col0 = rag_win[0][2]

    off = n0
    # per (class-k, core): number of real doms; and split into full/ragged
    cls_meta = []  # (k, m, n_real, fw, r)
    for k in range(1, kmax + 1):
        if n_k[k] == 0:
            continue
        m = -(-int(n_k[k]) // NCORES)
        fw = m // 128
        r = m % 128
        n_real = int(n_k[k])
        doms_k = dom_order[off : off + n_real]
        off += n_real
        tot = NCORES * m
        core_of = np.arange(tot, dtype=np.int32) % NCORES
        pos_of = np.arange(tot, dtype=np.int32) // NCORES
        # column for position p: in full region if p < fw*128 else ragged
        col_full0 = full_map[k][2] if fw else 0
        p = pos_of[:n_real]
        cols = np.where(
            p < fw * 128,
            col_full0 + p,
            rag_col0 + rag_off.get(k, 0) + (p - fw * 128),
        ).astype(np.int32)
        dom_core[doms_k] = core_of[:n_real]
        dom_col[doms_k] = cols
        cls_meta.append((k, m, n_real, fw, r))

    # pulses sorted by (core, dom column)
    key = dom_core[idx].astype(np.int64) * (1 << 32) + dom_col[idx]
    perm = np.argsort(key, kind="stable")
    core_pulse_counts = np.bincount(dom_core[idx], minlength=NCORES)
    core_splits = np.concatenate([[0], np.cumsum(core_pulse_counts)])

    # ragged window lookup per ragged position
    if nrw:
        rag_kw = np.concatenate(
            [np.full(128, kw, np.int32) for kw, _, _ in rag_win]
        )[: nrw * 128]
        rag_base = np.array([bse for _, bse, _ in rag_win], np.int64)

    # ---- build per-core slot buffers ------------------------------------
    bufs = []
    for c in range(NCORES):
        buf = np.zeros(s_elems, np.float32)
        pc = perm[core_splits[c] : core_splits[c + 1]]
        p_off = 0
        # pass 1: full-window regions, in column order (= ascending k)
        for k, m, n_real, fw, r in cls_meta:
            nreal_c = n_real // NCORES + (1 if c < n_real % NCORES else 0)
            n_full = min(nreal_c, fw * 128)
            if n_full == 0:
                continue
            R_rows = pc[p_off : p_off + n_full * k].reshape(n_full, k)
            p_off += n_full * k
            base = full_map[k][3]
            A = E[R_rows].transpose(0, 2, 1)  # (n, D, k)
            buf[base : base + n_full * D * k] = A.reshape(-1)
        # pass 2: ragged region, in column order (= ascending k)
        for k, m, n_real, fw, r in cls_meta:
            nreal_c = n_real // NCORES + (1 if c < n_real % NCORES else 0)
            n_full = min(nreal_c, fw * 128)
            n_rag = nreal_c - n_full
            if n_rag == 0:
                continue
            R_rows = pc[p_off : p_off + n_rag * k].reshape(n_rag, k)
            p_off += n_rag * k
            rp0 = rag_off[k]
            Arag = E[R_rows].transpose(0, 2, 1)  # (n_rag, D, k)
            i = 0
            while i < n_rag:
                rp = rp0 + i
                j = rp // 128
                kw = int(rag_kw[rp])
                lim = min(n_rag, (j + 1) * 128 - rp0)  # same-window chunk
                chunk = Arag[i:lim]                    # (cn, D, k)
                cn = chunk.shape[0]
                blk = np.empty((cn, D, kw), np.float32)
                blk[:, :, :k] = chunk
                if kw > k:
                    blk[:, :, k:] = chunk[:, :, 0:1]
                bse = int(rag_base[j]) + (rp - j * 128) * D * kw
                buf[bse : bse + cn * D * kw] = blk.reshape(-1)
                i = lim
        bufs.append(buf)

    # ---- weights / tables ----------------------------------------------
    WT = np.ascontiguousarray(W.T)  # (2D, D)
    wts = np.empty((nwt * 128, D), np.float32)
    for jc, (k, fw, col0, base) in enumerate(full_cls):
        blk = WT.copy()
        blk[0:D] *= np.float32(1.0 / k)
        wts[jc * 128 : (jc + 1) * 128] = blk
    wts[len(full_cls) * 128 :] = WT  # unscaled for ragged
    b_col = b.reshape(D, 1)

    rt = None
    if nrw:
        rt = np.zeros((nrw * 128, 2), np.float32)
        rt[:, 0] = 1.0
        kd = np.zeros(nrw * 128, np.int32)
        pos = 0
        for k, r in rag_cls:
            kd[pos : pos + r] = k
            pos += r
        real = kd > 0
        rt[real, 0] = 1.0 / kd[real]
        rt[real, 1] = (rag_kw[real] - kd[real]).astype(np.float32)

    # ---- device ---------------------------------------------------------
    nc = _build_nc(full_cls, rag_win, ndcp, s_elems, nwt, nrw)
    in_maps = []
    for c in range(NCORES):
        m = {"slots": bufs[c], "wts": wts, "b": b_col}
        if nrw:
            m["rt"] = rt
        in_maps.append(m)
    trace = os.environ.get("KERNEL_TRACE", "0") == "1"
    kw_ = {}
    if trace:
        import tempfile
        kw_ = dict(trace=True, tmpdir=tempfile.mkdtemp(prefix="kernel_trace_"))
    res = run_bass_kernel_spmd(nc, in_maps, core_ids=list(range(NCORES)), **kw_)
    last_exec_ns = res.exec_time_ns

    # ---- host-side unpermute -------------------------------------------
    outs = np.stack([res.results[c]["out"] for c in range(NCORES)])  # (8, D, ndcp)
    full = np.empty((nd, D), np.float32)
    real = dom_core >= 0
    full[real] = outs[dom_core[real], :, dom_col[real]]
    if n0:
        full[~real] = b
    return full



# revision 12
# speedup vs baseline: 1.0132x; 1.0132x over previous
"""DOM pooling (segment mean+max over pulses, then linear projection) on 8 trn2 cores.

Strategy:
  Host: bucket DOMs by exact pulse count k ("classes"); deal DOMs of each class
  round-robin across the 8 cores so every core has an identical class structure
  (same per-class DOM count m_k, padded with dummy DOMs). Full 128-DOM windows
  are emitted per class; the leftover (<128) DOMs of every class are packed
  together into shared "ragged" windows (sorted by k, per-DOM slots padded to
  the window capacity by duplicating the DOM's first slot — max-neutral; the
  sum is corrected on device by subtracting padcnt*x0 and scaled by 1/k).
  Each core gets a gathered slot buffer where a DOM's k pulse embeddings are
  stored embed-major (64 x k contiguous), so device reads are sequential.

  Device (one NEFF, SPMD on 8 cores), per 128-DOM window:
    - batched contiguous DMA loads
    - DVE reduce_sum / reduce_max over the slots (contiguous inner axis)
    - PE transpose of [sum|max] concat (128x128) -> PSUM (feat-major)
    - projection matmul out^T = Wk^T.T @ concatT (mean scaling 1/k folded into
      the sum-feature rows of the per-class weights; ragged windows scale on
      DVE and use unscaled weights)
    - ACT adds bias during PSUM->SBUF copy; batched DMA writes out^T.

  Host: scatter per-core transposed outputs back to the full (num_doms, 64).
"""
import sys

import numpy as np

for _p in ("/opt/trn_rl_repo",):
    if _p not in sys.path:
        sys.path.append(_p)

from concourse import bacc
import concourse.mybir as mybir
import concourse.tile as tile
from concourse.bass_utils import run_bass_kernel_spmd
from concourse.masks import make_identity

NCORES = 8
D = 64
FP32 = mybir.dt.float32
FP16 = mybir.dt.float16

last_exec_ns = None  # set when KERNEL_TRACE=1


def _plan(counts):
    """Class/window structure shared by all cores (derived from global counts).

    Returns:
      full_cls: list of (k, fw, col0, base) classes with fw full windows
      rag_cls:  list of (k, r) leftover doms per class (class order)
      rag_win:  list of (k_w, base, col0) ragged windows
      ndcp:     output columns per core
      s_elems:  slot buffer elements per core
    """
    kmax = int(counts.max()) if counts.size else 0
    n_k = np.bincount(counts, minlength=kmax + 1)
    full_cls = []
    rag_cls = []
    col = 0
    slot = 0
    for k in range(1, kmax + 1):
        if n_k[k] == 0:
            continue
        m = -(-int(n_k[k]) // NCORES)
        fw = m // 128
        r = m % 128
        if fw:
            full_cls.append((k, fw, col, slot))
            col += fw * 128
            slot += fw * 128 * k * D
        if r:
            rag_cls.append((k, r))
    # ragged windows: doms in class order (ascending k); capacity = max k in win
    rag_win = []
    R = sum(r for _, r in rag_cls)
    if R:
        ks = np.concatenate([np.full(r, k, np.int32) for k, r in rag_cls])
        RW = -(-R // 128)
        for j in range(RW):
            seg = ks[j * 128 : (j + 1) * 128]
            kw = int(seg.max())
            rag_win.append((kw, slot, col + j * 128))
            slot += 128 * kw * D
        col += RW * 128
    return full_cls, rag_cls, rag_win, col, slot


def _build_nc(full_cls, rag_win, ndcp, s_elems, nwt, nrw):
    nc = bacc.Bacc(None)
    slots_t = nc.dram_tensor("slots", [s_elems], FP16, kind="ExternalInput")
    wts_t = nc.dram_tensor("wts", [nwt * 128, D], FP16, kind="ExternalInput")
    b_t = nc.dram_tensor("b", [D, 1], FP32, kind="ExternalInput")
    if nrw:
        rt_t = nc.dram_tensor("rt", [nrw * 128, 2], FP32, kind="ExternalInput")
    out_t = nc.dram_tensor("out", [D, ndcp], FP32, kind="ExternalOutput")

    with tile.TileContext(nc) as tc:
        with (
            nc.allow_low_precision(
                reason="fp16 segment sums: |sum|<=~250, k<=~64; rel tol 2e-2"
            ),
            tc.tile_pool(name="const", bufs=1) as constp,
            tc.tile_pool(name="inp", bufs=6) as inp,
            tc.tile_pool(name="mid", bufs=4) as midp,
            tc.tile_pool(name="outp", bufs=4) as outp,
            tc.tile_pool(name="psA", bufs=4, space="PSUM") as psA,
            tc.tile_pool(name="psB", bufs=4, space="PSUM") as psB,
        ):
            ident = constp.tile([128, 128], FP16)
            make_identity(nc, ident[:])
            wt_sb = constp.tile([128, nwt * D], FP16)
            nc.sync.dma_start(
                wt_sb[:].rearrange("p (j e) -> p j e", e=D),
                wts_t[:, :].rearrange("(j p) e -> p j e", p=128),
            )
            b_sb = constp.tile([D, 1], FP32)
            nc.sync.dma_start(b_sb[:], b_t[:])
            if nrw:
                rt_sb = constp.tile([128, nrw * 2], FP32)
                nc.sync.dma_start(
                    rt_sb[:].rearrange("p (j c) -> p j c", c=2),
                    rt_t[:, :].rearrange("(j p) c -> p j c", p=128),
                )

            def window_tail(cat, out_ap, p, jwt):
                """cat: (p, 128) [sum|max] slice; out_ap: (D, p) slice."""
                catT_ps = psA.tile([128, 128], FP16, space="PSUM", tag="ps")
                nc.tensor.transpose(
                    out=catT_ps[:, :p], in_=cat, identity=ident[:p, :p]
                )
                catT = midp.tile([128, 128], FP16, tag="catT")
                nc.scalar.copy(catT[:, :p], catT_ps[:, :p])
                proj_ps = psB.tile([D, 128], FP32, space="PSUM", tag="proj")
                nc.tensor.matmul(
                    proj_ps[:, :p],
                    lhsT=wt_sb[:, jwt * D : (jwt + 1) * D],
                    rhs=catT[:, :p],
                    start=True,
                    stop=True,
                )
                nc.scalar.activation(
                    out_ap, proj_ps[:, :p],
                    mybir.ActivationFunctionType.Identity, bias=b_sb[:, :1],
                )

            # ---- full per-class windows --------------------------------
            for jcls, (k, fw, col0, base) in enumerate(full_cls):
                G = max(1, min(8, 128 // k))
                g = 0
                while g < fw:
                    gw = min(G, fw - g)
                    in_t = inp.tile([128, gw * k * D], FP16, tag="in")
                    src = slots_t[
                        base + g * 128 * k * D : base + (g + gw) * 128 * k * D
                    ].rearrange("(w d f) -> d w f", w=gw, d=128)
                    nc.sync.dma_start(
                        in_t[:].rearrange("d (w f) -> d w f", w=gw), src
                    )
                    cat_g = midp.tile([128, 8 * 128], FP16, tag="cat")
                    co = cat_g[:, : gw * 128].rearrange("d (w c) -> d w c", c=128)
                    if k == 1:
                        v = in_t[:].rearrange("d (w e) -> d w e", w=gw)
                        nc.vector.tensor_copy(co[:, :, 0:D], v)
                        nc.vector.tensor_copy(co[:, :, D:128], v)
                    else:
                        view = in_t[:].rearrange("d (w e s) -> d w e s", w=gw, s=k)
                        nc.vector.reduce_sum(co[:, :, 0:D], view, axis=mybir.AxisListType.X)
                        nc.vector.reduce_max(co[:, :, D:128], view, axis=mybir.AxisListType.X)
                    out_sb = outp.tile([64, 8 * 128], FP32, tag="out")
                    for w in range(gw):
                        window_tail(
                            cat_g[:, w * 128 : (w + 1) * 128],
                            out_sb[:, w * 128 : (w + 1) * 128],
                            128, jcls,
                        )
                    nc.sync.dma_start(
                        out_t[:, col0 + g * 128 : col0 + (g + gw) * 128],
                        out_sb[:, : gw * 128],
                    )
                    g += gw

            # ---- ragged windows (mixed k, capacity k_w) ----------------
            juns = len(full_cls)  # unscaled weight block index
            for j, (kw, base, col0) in enumerate(rag_win):
                in_t = inp.tile([128, kw * D], FP16, tag="in")
                nc.sync.dma_start(
                    in_t[:], slots_t[base : base + 128 * kw * D].rearrange(
                        "(d f) -> d f", d=128
                    ),
                )
                cat_g = midp.tile([128, 8 * 128], FP16, tag="cat")
                view = in_t[:].rearrange("d (e s) -> d e s", s=kw)
                sraw = midp.tile([128, D], FP16, tag="sraw")
                nc.vector.reduce_sum(sraw[:], view, axis=mybir.AxisListType.X)
                nc.vector.reduce_max(cat_g[:, D:128], view, axis=mybir.AxisListType.X)
                # sum correction: (sraw - padcnt*x0) * recip_k
                x0 = view[:, :, 0]
                tmp = midp.tile([128, D], FP16, tag="tmp")
                nc.vector.tensor_scalar_mul(
                    tmp[:], x0, rt_sb[:, j * 2 + 1 : j * 2 + 2]
                )
                nc.vector.tensor_tensor(
                    out=tmp[:], in0=sraw[:], in1=tmp[:], op=mybir.AluOpType.subtract
                )
                nc.vector.tensor_scalar_mul(
                    cat_g[:, 0:D], tmp[:], rt_sb[:, j * 2 : j * 2 + 1]
                )
                out_sb = outp.tile([64, 8 * 128], FP32, tag="out")
                window_tail(cat_g[:, 0:128], out_sb[:, :128], 128, juns)
                nc.sync.dma_start(out_t[:, col0 : col0 + 128], out_sb[:, :128])
    nc.finalize()
    return nc


def kernel(pulse_embeddings, pulse_to_dom_idx, num_doms, proj_w, proj_b):
    global last_exec_ns
    import os

    E = np.ascontiguousarray(np.asarray(pulse_embeddings, dtype=np.float32)).astype(
        np.float16
    )
    idx = np.asarray(pulse_to_dom_idx).astype(np.int64)
    nd = int(num_doms)
    W = np.asarray(proj_w, dtype=np.float32)   # (D, 2D)
    b = np.asarray(proj_b, dtype=np.float32)   # (D,)

    counts = np.bincount(idx, minlength=nd)
    full_cls, rag_cls, rag_win, ndcp, s_elems = _plan(counts)
    nwt = len(full_cls) + 1
    nrw = len(rag_win)

    # ---- host-side dom assignment --------------------------------------
    dom_order = np.argsort(counts, kind="stable")
    cs = counts[dom_order]
    n0 = int((counts == 0).sum())
    dom_core = np.full(nd, -1, np.int32)
    dom_col = np.full(nd, -1, np.int32)

    # per-class bookkeeping (shared across cores)
    kmax = int(counts.max()) if counts.size else 0
    n_k = np.bincount(counts, minlength=kmax + 1)
    full_map = {k: (jc, fw, col0, base) for jc, (k, fw, col0, base) in enumerate(full_cls)}
    # ragged: position of each class's leftover run inside the ragged region
    rag_off = {}
    ro = 0
    for k, r in rag_cls:
        rag_off[k] = ro
        ro += r
    R = ro
    rag_col0 = rag_win[0][2] - 0 if rag_win else ndcp  # col of ragged dom 0
    if rag_win:
        rag_col0 = rag_win[0][2]

    off = n0
    # per (class-k, core): number of real doms; and split into full/ragged
    cls_meta = []  # (k, m, n_real, fw, r)
    for k in range(1, kmax + 1):
        if n_k[k] == 0:
            continue
        m = -(-int(n_k[k]) // NCORES)
        fw = m // 128
        r = m % 128
        n_real = int(n_k[k])
        doms_k = dom_order[off : off + n_real]
        off += n_real
        tot = NCORES * m
        core_of = np.arange(tot, dtype=np.int32) % NCORES
        pos_of = np.arange(tot, dtype=np.int32) // NCORES
        # column for position p: in full region if p < fw*128 else ragged
        col_full0 = full_map[k][2] if fw else 0
        p = pos_of[:n_real]
        cols = np.where(
            p < fw * 128,
            col_full0 + p,
            rag_col0 + rag_off.get(k, 0) + (p - fw * 128),
        ).astype(np.int32)
        dom_core[doms_k] = core_of[:n_real]
        dom_col[doms_k] = cols
        cls_meta.append((k, m, n_real, fw, r))

    # pulses sorted by (core, dom column)
    key = dom_core[idx].astype(np.int64) * (1 << 32) + dom_col[idx]
    perm = np.argsort(key, kind="stable")
    core_pulse_counts = np.bincount(dom_core[idx], minlength=NCORES)
    core_splits = np.concatenate([[0], np.cumsum(core_pulse_counts)])

    # ragged window lookup per ragged position
    if nrw:
        rag_kw = np.concatenate(
            [np.full(128, kw, np.int32) for kw, _, _ in rag_win]
        )[: nrw * 128]
        rag_base = np.array([bse for _, bse, _ in rag_win], np.int64)

    # ---- build per-core slot buffers ------------------------------------
    bufs = []
    for c in range(NCORES):
        buf = np.zeros(s_elems, np.float16)
        pc = perm[core_splits[c] : core_splits[c + 1]]
        p_off = 0
        # pass 1: full-window regions, in column order (= ascending k)
        for k, m, n_real, fw, r in cls_meta:
            nreal_c = n_real // NCORES + (1 if c < n_real % NCORES else 0)
            n_full = min(nreal_c, fw * 128)
            if n_full == 0:
                continue
            R_rows = pc[p_off : p_off + n_full * k].reshape(n_full, k)
            p_off += n_full * k
            base = full_map[k][3]
            A = E[R_rows].transpose(0, 2, 1)  # (n, D, k)
            buf[base : base + n_full * D * k] = A.reshape(-1)
        # pass 2: ragged region, in column order (= ascending k)
        for k, m, n_real, fw, r in cls_meta:
            nreal_c = n_real // NCORES + (1 if c < n_real % NCORES else 0)
            n_full = min(nreal_c, fw * 128)
            n_rag = nreal_c - n_full
            if n_rag == 0:
                continue
            R_rows = pc[p_off : p_off + n_rag * k].reshape(n_rag, k)
            p_off += n_rag * k
            rp0 = rag_off[k]
            Arag = E[R_rows].transpose(0, 2, 1)  # (n_rag, D, k)
            i = 0
            while i < n_rag:
                rp = rp0 + i
                j = rp // 128
                kw = int(rag_kw[rp])
                lim = min(n_rag, (j + 1) * 128 - rp0)  # same-window chunk
                chunk = Arag[i:lim]                    # (cn, D, k)
                cn = chunk.shape[0]
                blk = np.empty((cn, D, kw), np.float16)
                blk[:, :, :k] = chunk
                if kw > k:
                    blk[:, :, k:] = chunk[:, :, 0:1]
                bse = int(rag_base[j]) + (rp - j * 128) * D * kw
                buf[bse : bse + cn * D * kw] = blk.reshape(-1)
                i = lim
        bufs.append(buf)

    # ---- weights / tables ----------------------------------------------
    WT = np.ascontiguousarray(W.T)  # (2D, D)
    wts = np.empty((nwt * 128, D), np.float16)
    for jc, (k, fw, col0, base) in enumerate(full_cls):
        blk = WT.copy()
        blk[0:D] *= np.float32(1.0 / k)
        wts[jc * 128 : (jc + 1) * 128] = blk
    wts[len(full_cls) * 128 :] = WT  # unscaled for ragged
    b_col = b.reshape(D, 1)

    rt = None
    if nrw:
        rt = np.zeros((nrw * 128, 2), np.float32)
        rt[:, 0] = 1.0
        kd = np.zeros(nrw * 128, np.int32)
        pos = 0
        for k, r in rag_cls:
            kd[pos : pos + r] = k
            pos += r
        real = kd > 0
        rt[real, 0] = 1.0 / kd[real]
        rt[real, 1] = (rag_kw[real] - kd[real]).astype(np.float32)

    # ---- device ---------------------------------------------------------
    nc = _build_nc(full_cls, rag_win, ndcp, s_elems, nwt, nrw)
    in_maps = []
    for c in range(NCORES):
        m = {"slots": bufs[c], "wts": wts, "b": b_col}
        if nrw:
            m["rt"] = rt
        in_maps.append(m)
    trace = os.environ.get("KERNEL_TRACE", "0") == "1"
    kw_ = {}
    if trace:
        import tempfile
        kw_ = dict(trace=True, tmpdir=tempfile.mkdtemp(prefix="kernel_trace_"))
    res = run_bass_kernel_spmd(nc, in_maps, core_ids=list(range(NCORES)), **kw_)
    last_exec_ns = res.exec_time_ns

    # ---- host-side unpermute -------------------------------------------
    outs = np.stack([res.results[c]["out"] for c in range(NCORES)])  # (8, D, ndcp)
    full = np.empty((nd, D), np.float32)
    real = dom_core >= 0
    full[real] = outs[dom_core[real], :, dom_col[real]]
    if n0:
        full[~real] = b
    return full



# revision 17
# speedup vs baseline: 1.6390x; 1.6176x over previous
"""DOM pooling (segment mean+max over pulses, then linear projection) on 8 trn2 cores.

Strategy:
  Host: bucket DOMs by exact pulse count k ("classes"); deal DOMs of each class
  round-robin across the 8 cores so every core has an identical class structure
  (same per-class DOM count m_k, padded with dummy DOMs). Full 128-DOM windows
  are emitted per class; the leftover (<128) DOMs of every class are packed
  together into shared "ragged" windows (sorted by k, per-DOM slots padded to
  the window capacity by duplicating the DOM's first slot — max-neutral; the
  sum is corrected on device by subtracting padcnt*x0 and scaled by 1/k).
  Each core gets a gathered slot buffer where a DOM's k pulse embeddings are
  stored embed-major (64 x k contiguous), so device reads are sequential.

  Device (one NEFF, SPMD on 8 cores), per 128-DOM window:
    - batched contiguous DMA loads
    - DVE reduce_sum / reduce_max over the slots (contiguous inner axis)
    - PE transpose of [sum|max] concat (128x128) -> PSUM (feat-major)
    - projection matmul out^T = Wk^T.T @ concatT (mean scaling 1/k folded into
      the sum-feature rows of the per-class weights; ragged windows scale on
      DVE and use unscaled weights)
    - ACT adds bias during PSUM->SBUF copy; batched DMA writes out^T.

  Host: scatter per-core transposed outputs back to the full (num_doms, 64).
"""
import sys

import numpy as np

for _p in ("/opt/trn_rl_repo",):
    if _p not in sys.path:
        sys.path.append(_p)

from concourse import bacc
import concourse.mybir as mybir
import concourse.tile as tile
from concourse.bass_utils import run_bass_kernel_spmd
from concourse.masks import make_identity

NCORES = 8
D = 64
FP32 = mybir.dt.float32
FP16 = mybir.dt.float16

last_exec_ns = None  # set when KERNEL_TRACE=1


def _plan(counts):
    """Class/window structure shared by all cores (derived from global counts).

    Returns:
      full_cls: list of (k, fw, col0, base) classes with fw full windows
      rag_cls:  list of (k, r) leftover doms per class (class order)
      rag_win:  list of (k_w, base, col0) ragged windows
      ndcp:     output columns per core
      s_elems:  slot buffer elements per core
    """
    kmax = int(counts.max()) if counts.size else 0
    n_k = np.bincount(counts, minlength=kmax + 1)
    full_cls = []
    rag_cls = []
    col = 0
    slot = 0
    for k in range(1, kmax + 1):
        if n_k[k] == 0:
            continue
        m = -(-int(n_k[k]) // NCORES)
        fw = m // 128
        r = m % 128
        if fw:
            full_cls.append((k, fw, col, slot))
            col += fw * 128
            slot += fw * 128 * k * D
        if r:
            rag_cls.append((k, r))
    # ragged windows: doms in class order (ascending k); capacity = max k in win
    rag_win = []
    R = sum(r for _, r in rag_cls)
    if R:
        ks = np.concatenate([np.full(r, k, np.int32) for k, r in rag_cls])
        RW = -(-R // 128)
        for j in range(RW):
            seg = ks[j * 128 : (j + 1) * 128]
            kw = int(seg.max())
            rag_win.append((kw, slot, col + j * 128))
            slot += 128 * kw * D
        col += RW * 128
    return full_cls, rag_cls, rag_win, col, slot


def _build_nc(full_cls, rag_win, ndcp, s_elems, nwt, nrw):
    nc = bacc.Bacc(None)
    slots_t = nc.dram_tensor("slots", [s_elems], FP16, kind="ExternalInput")
    wts_t = nc.dram_tensor("wts", [nwt * 128, D], FP16, kind="ExternalInput")
    b_t = nc.dram_tensor("b", [D, 1], FP32, kind="ExternalInput")
    if nrw:
        rt_t = nc.dram_tensor("rt", [nrw * 128, 2], FP32, kind="ExternalInput")
    out_t = nc.dram_tensor("out", [D, ndcp], FP32, kind="ExternalOutput")

    with tile.TileContext(nc) as tc:
        with (
            nc.allow_low_precision(
                reason="fp16 segment sums: |sum|<=~250, k<=~64; rel tol 2e-2"
            ),
            tc.tile_pool(name="const", bufs=1) as constp,
            tc.tile_pool(name="inp", bufs=4) as inp,
            tc.tile_pool(name="scr", bufs=2) as scrp,
            tc.tile_pool(name="mid", bufs=4) as midp,
            tc.tile_pool(name="outp", bufs=4) as outp,
            tc.tile_pool(name="psA", bufs=4, space="PSUM") as psA,
            tc.tile_pool(name="psB", bufs=4, space="PSUM") as psB,
        ):
            # scratch col sizes per fold-tree level (fp16 elems)
            SCR_SZ = [4608, 2560, 1536, 1024, 640, 384]
            ident = constp.tile([128, 128], FP16)
            make_identity(nc, ident[:])
            wt_sb = constp.tile([128, nwt * D], FP16)
            nc.sync.dma_start(
                wt_sb[:].rearrange("p (j e) -> p j e", e=D),
                wts_t[:, :].rearrange("(j p) e -> p j e", p=128),
            )
            b_sb = constp.tile([D, 1], FP32)
            nc.sync.dma_start(b_sb[:], b_t[:])
            if nrw:
                rt_sb = constp.tile([128, nrw * 2], FP32)
                nc.sync.dma_start(
                    rt_sb[:].rearrange("p (j c) -> p j c", c=2),
                    rt_t[:, :].rearrange("(j p) c -> p j c", p=128),
                )

            def window_tail(cat, out_ap, p, jwt):
                """cat: (p, 128) [sum|max] slice; out_ap: (D, p) slice."""
                catT_ps = psA.tile([128, 128], FP16, space="PSUM", tag="ps")
                nc.tensor.transpose(
                    out=catT_ps[:, :p], in_=cat, identity=ident[:p, :p]
                )
                catT = midp.tile([128, 128], FP16, tag="catT")
                nc.scalar.copy(catT[:, :p], catT_ps[:, :p])
                proj_ps = psB.tile([D, 128], FP32, space="PSUM", tag="proj")
                nc.tensor.matmul(
                    proj_ps[:, :p],
                    lhsT=wt_sb[:, jwt * D : (jwt + 1) * D],
                    rhs=catT[:, :p],
                    start=True,
                    stop=True,
                )
                nc.scalar.activation(
                    out_ap, proj_ps[:, :p],
                    mybir.ActivationFunctionType.Identity, bias=b_sb[:, :1],
                )

            # ---- full per-class windows --------------------------------
            # Slot-outer layout per group: free = (s, w, e). Pairwise fold
            # trees on DVE tensor_tensor (fp16 2x mode: ~0.52ns/out-elem vs
            # reduce's 1.06ns/in-elem). Every fold reads two contiguous
            # halves; odd levels carry the leftover slot via tensor_copy
            # (4x mode).
            def fold_tree(in_ap, k, U, op, out_ap, tag):
                """in_ap: [128, k*U] contiguous; out_ap: [128, gw, 64] view
                (strided over w allowed, innermost 64 packed)."""
                m, cur, lvl = k, in_ap, 0
                while m > 1:
                    h, r = divmod(m, 2)
                    if (h + r) == 1:
                        nc.vector.tensor_tensor(
                            out=out_ap,
                            in0=cur[:, :U].rearrange("d (w e) -> d w e", e=D),
                            in1=cur[:, U : 2 * U].rearrange(
                                "d (w e) -> d w e", e=D
                            ),
                            op=op,
                        )
                        return
                    t = scrp.tile([128, SCR_SZ[lvl]], FP16, tag=f"{tag}{lvl}")
                    dst = t[:, : (h + r) * U]
                    nc.vector.tensor_tensor(
                        out=dst[:, : h * U],
                        in0=cur[:, : h * U],
                        in1=cur[:, h * U : 2 * h * U],
                        op=op,
                    )
                    if r:
                        nc.vector.tensor_copy(
                            dst[:, h * U : (h + 1) * U],
                            cur[:, 2 * h * U : m * U],
                        )
                    cur = dst
                    m = h + r
                    lvl += 1

            for jcls, (k, fw, col0, base) in enumerate(full_cls):
                G = max(1, min(8, 128 // k))
                g = 0
                while g < fw:
                    gw = min(G, fw - g)
                    U = gw * D
                    in_t = inp.tile([128, gw * k * D], FP16, tag="in")
                    nc.sync.dma_start(
                        in_t[:],
                        slots_t[
                            base + g * 128 * k * D : base + (g + gw) * 128 * k * D
                        ].rearrange("(d f) -> d f", d=128),
                    )
                    cat_g = midp.tile([128, 8 * 128], FP16, tag="cat")
                    co = cat_g[:, : gw * 128].rearrange("d (w c) -> d w c", c=128)
                    if k == 1:
                        v = in_t[:].rearrange("d (w e) -> d w e", w=gw)
                        nc.vector.tensor_copy(co[:, :, 0:D], v)
                        nc.vector.tensor_copy(co[:, :, D:128], v)
                    else:
                        fold_tree(
                            in_t[:], k, U, mybir.AluOpType.add,
                            co[:, :, 0:D], "s",
                        )
                        fold_tree(
                            in_t[:], k, U, mybir.AluOpType.max,
                            co[:, :, D:128], "m",
                        )
                    out_sb = outp.tile([64, 8 * 128], FP32, tag="out")
                    for w in range(gw):
                        window_tail(
                            cat_g[:, w * 128 : (w + 1) * 128],
                            out_sb[:, w * 128 : (w + 1) * 128],
                            128, jcls,
                        )
                    nc.sync.dma_start(
                        out_t[:, col0 + g * 128 : col0 + (g + gw) * 128],
                        out_sb[:, : gw * 128],
                    )
                    g += gw

            # ---- ragged windows (mixed k, capacity k_w) ----------------
            juns = len(full_cls)  # unscaled weight block index
            for j, (kw, base, col0) in enumerate(rag_win):
                in_t = inp.tile([128, kw * D], FP16, tag="in")
                nc.sync.dma_start(
                    in_t[:], slots_t[base : base + 128 * kw * D].rearrange(
                        "(d f) -> d f", d=128
                    ),
                )
                cat_g = midp.tile([128, 8 * 128], FP16, tag="cat")
                view = in_t[:].rearrange("d (e s) -> d e s", s=kw)
                sraw = midp.tile([128, D], FP16, tag="sraw")
                nc.vector.reduce_sum(sraw[:], view, axis=mybir.AxisListType.X)
                nc.vector.reduce_max(cat_g[:, D:128], view, axis=mybir.AxisListType.X)
                # sum correction: (sraw - padcnt*x0) * recip_k
                x0 = view[:, :, 0]
                tmp = midp.tile([128, D], FP16, tag="tmp")
                nc.vector.tensor_scalar_mul(
                    tmp[:], x0, rt_sb[:, j * 2 + 1 : j * 2 + 2]
                )
                nc.vector.tensor_tensor(
                    out=tmp[:], in0=sraw[:], in1=tmp[:], op=mybir.AluOpType.subtract
                )
                nc.vector.tensor_scalar_mul(
                    cat_g[:, 0:D], tmp[:], rt_sb[:, j * 2 : j * 2 + 1]
                )
                out_sb = outp.tile([64, 8 * 128], FP32, tag="out")
                window_tail(cat_g[:, 0:128], out_sb[:, :128], 128, juns)
                nc.sync.dma_start(out_t[:, col0 : col0 + 128], out_sb[:, :128])
    nc.finalize()
    return nc


def kernel(pulse_embeddings, pulse_to_dom_idx, num_doms, proj_w, proj_b):
    global last_exec_ns
    import os

    E = np.ascontiguousarray(np.asarray(pulse_embeddings, dtype=np.float32)).astype(
        np.float16
    )
    idx = np.asarray(pulse_to_dom_idx).astype(np.int64)
    nd = int(num_doms)
    W = np.asarray(proj_w, dtype=np.float32)   # (D, 2D)
    b = np.asarray(proj_b, dtype=np.float32)   # (D,)

    counts = np.bincount(idx, minlength=nd)
    full_cls, rag_cls, rag_win, ndcp, s_elems = _plan(counts)
    nwt = len(full_cls) + 1
    nrw = len(rag_win)

    # ---- host-side dom assignment --------------------------------------
    dom_order = np.argsort(counts, kind="stable")
    cs = counts[dom_order]
    n0 = int((counts == 0).sum())
    dom_core = np.full(nd, -1, np.int32)
    dom_col = np.full(nd, -1, np.int32)

    # per-class bookkeeping (shared across cores)
    kmax = int(counts.max()) if counts.size else 0
    n_k = np.bincount(counts, minlength=kmax + 1)
    full_map = {k: (jc, fw, col0, base) for jc, (k, fw, col0, base) in enumerate(full_cls)}
    # ragged: position of each class's leftover run inside the ragged region
    rag_off = {}
    ro = 0
    for k, r in rag_cls:
        rag_off[k] = ro
        ro += r
    R = ro
    rag_col0 = rag_win[0][2] - 0 if rag_win else ndcp  # col of ragged dom 0
    if rag_win:
        rag_col0 = rag_win[0][2]

    off = n0
    # per (class-k, core): number of real doms; and split into full/ragged
    cls_meta = []  # (k, m, n_real, fw, r)
    for k in range(1, kmax + 1):
        if n_k[k] == 0:
            continue
        m = -(-int(n_k[k]) // NCORES)
        fw = m // 128
        r = m % 128
        n_real = int(n_k[k])
        doms_k = dom_order[off : off + n_real]
        off += n_real
        tot = NCORES * m
        core_of = np.arange(tot, dtype=np.int32) % NCORES
        pos_of = np.arange(tot, dtype=np.int32) // NCORES
        # column for position p: in full region if p < fw*128 else ragged
        col_full0 = full_map[k][2] if fw else 0
        p = pos_of[:n_real]
        cols = np.where(
            p < fw * 128,
            col_full0 + p,
            rag_col0 + rag_off.get(k, 0) + (p - fw * 128),
        ).astype(np.int32)
        dom_core[doms_k] = core_of[:n_real]
        dom_col[doms_k] = cols
        cls_meta.append((k, m, n_real, fw, r))

    # pulses sorted by (core, dom column)
    key = dom_core[idx].astype(np.int64) * (1 << 32) + dom_col[idx]
    perm = np.argsort(key, kind="stable")
    core_pulse_counts = np.bincount(dom_core[idx], minlength=NCORES)
    core_splits = np.concatenate([[0], np.cumsum(core_pulse_counts)])

    # ragged window lookup per ragged position
    if nrw:
        rag_kw = np.concatenate(
            [np.full(128, kw, np.int32) for kw, _, _ in rag_win]
        )[: nrw * 128]
        rag_base = np.array([bse for _, bse, _ in rag_win], np.int64)

    # ---- build per-core slot buffers ------------------------------------
    bufs = []
    for c in range(NCORES):
        buf = np.zeros(s_elems, np.float16)
        pc = perm[core_splits[c] : core_splits[c + 1]]
        p_off = 0
        # pass 1: full-window regions, in column order (= ascending k).
        # Group blocks stored slot-outer: [128 doms][slot][window][feat] so
        # the device fold-tree reads contiguous halves at every level.
        for k, m, n_real, fw, r in cls_meta:
            nreal_c = n_real // NCORES + (1 if c < n_real % NCORES else 0)
            n_full = min(nreal_c, fw * 128)
            if fw == 0 or n_full == 0:
                continue
            R_rows = pc[p_off : p_off + n_full * k].reshape(n_full, k)
            p_off += n_full * k
            base = full_map[k][3]
            Z = np.zeros((fw * 128, k, D), np.float16)
            Z[:n_full] = E[R_rows]
            G = max(1, min(8, 128 // k))
            g = 0
            while g < fw:
                gw = min(G, fw - g)
                blk = Z[g * 128 : (g + gw) * 128].reshape(gw, 128, k, D)
                off = base + g * 128 * k * D
                buf[off : off + gw * 128 * k * D] = blk.transpose(
                    1, 2, 0, 3
                ).ravel()
                g += gw
        # pass 2: ragged region, in column order (= ascending k)
        for k, m, n_real, fw, r in cls_meta:
            nreal_c = n_real // NCORES + (1 if c < n_real % NCORES else 0)
            n_full = min(nreal_c, fw * 128)
            n_rag = nreal_c - n_full
            if n_rag == 0:
                continue
            R_rows = pc[p_off : p_off + n_rag * k].reshape(n_rag, k)
            p_off += n_rag * k
            rp0 = rag_off[k]
            Arag = E[R_rows].transpose(0, 2, 1)  # (n_rag, D, k)
            i = 0
            while i < n_rag:
                rp = rp0 + i
                j = rp // 128
                kw = int(rag_kw[rp])
                lim = min(n_rag, (j + 1) * 128 - rp0)  # same-window chunk
                chunk = Arag[i:lim]                    # (cn, D, k)
                cn = chunk.shape[0]
                blk = np.empty((cn, D, kw), np.float16)
                blk[:, :, :k] = chunk
                if kw > k:
                    blk[:, :, k:] = chunk[:, :, 0:1]
                bse = int(rag_base[j]) + (rp - j * 128) * D * kw
                buf[bse : bse + cn * D * kw] = blk.reshape(-1)
                i = lim
        bufs.append(buf)

    # ---- weights / tables ----------------------------------------------
    WT = np.ascontiguousarray(W.T)  # (2D, D)
    wts = np.empty((nwt * 128, D), np.float16)
    for jc, (k, fw, col0, base) in enumerate(full_cls):
        blk = WT.copy()
        blk[0:D] *= np.float32(1.0 / k)
        wts[jc * 128 : (jc + 1) * 128] = blk
    wts[len(full_cls) * 128 :] = WT  # unscaled for ragged
    b_col = b.reshape(D, 1)

    rt = None
    if nrw:
        rt = np.zeros((nrw * 128, 2), np.float32)
        rt[:, 0] = 1.0
        kd = np.zeros(nrw * 128, np.int32)
        pos = 0
        for k, r in rag_cls:
            kd[pos : pos + r] = k
            pos += r
        real = kd > 0
        rt[real, 0] = 1.0 / kd[real]
        rt[real, 1] = (rag_kw[real] - kd[real]).astype(np.float32)

    # ---- device ---------------------------------------------------------
    nc = _build_nc(full_cls, rag_win, ndcp, s_elems, nwt, nrw)
    in_maps = []
    for c in range(NCORES):
        m = {"slots": bufs[c], "wts": wts, "b": b_col}
        if nrw:
            m["rt"] = rt
        in_maps.append(m)
    trace = os.environ.get("KERNEL_TRACE", "0") == "1"
    kw_ = {}
    if trace:
        import tempfile
        kw_ = dict(trace=True, tmpdir=tempfile.mkdtemp(prefix="kernel_trace_"))
    res = run_bass_kernel_spmd(nc, in_maps, core_ids=list(range(NCORES)), **kw_)
    last_exec_ns = res.exec_time_ns

    # ---- host-side unpermute -------------------------------------------
    outs = np.stack([res.results[c]["out"] for c in range(NCORES)])  # (8, D, ndcp)
    full = np.empty((nd, D), np.float32)
    real = dom_core >= 0
    full[real] = outs[dom_core[real], :, dom_col[real]]
    if n0:
        full[~real] = b
    return full



# revision 21
# speedup vs baseline: 1.7879x; 1.0908x over previous
"""DOM pooling (segment mean+max over pulses, then linear projection) on 8 trn2 cores.

Strategy:
  Host: bucket DOMs by exact pulse count k ("classes"); deal DOMs of each class
  round-robin across the 8 cores so every core has an identical class structure
  (same per-class DOM count m_k, padded with dummy DOMs). Full 128-DOM windows
  are emitted per class; the leftover (<128) DOMs of every class are packed
  together into shared "ragged" windows (sorted by k, per-DOM slots padded to
  the window capacity by duplicating the DOM's first slot — max-neutral; the
  sum is corrected on device by subtracting padcnt*x0 and scaled by 1/k).
  Each core gets a gathered slot buffer where a DOM's k pulse embeddings are
  stored embed-major (64 x k contiguous), so device reads are sequential.

  Device (one NEFF, SPMD on 8 cores), per 128-DOM window:
    - batched contiguous DMA loads
    - DVE reduce_sum / reduce_max over the slots (contiguous inner axis)
    - PE transpose of [sum|max] concat (128x128) -> PSUM (feat-major)
    - projection matmul out^T = Wk^T.T @ concatT (mean scaling 1/k folded into
      the sum-feature rows of the per-class weights; ragged windows scale on
      DVE and use unscaled weights)
    - ACT adds bias during PSUM->SBUF copy; batched DMA writes out^T.

  Host: scatter per-core transposed outputs back to the full (num_doms, 64).
"""
import sys

import numpy as np

for _p in ("/opt/trn_rl_repo",):
    if _p not in sys.path:
        sys.path.append(_p)

from concourse import bacc
import concourse.mybir as mybir
import concourse.tile as tile
from concourse.bass_utils import run_bass_kernel_spmd
from concourse.masks import make_identity

NCORES = 8
D = 64
FP32 = mybir.dt.float32
FP16 = mybir.dt.float16

last_exec_ns = None  # set when KERNEL_TRACE=1


def _plan(counts):
    """Class/window structure shared by all cores (derived from global counts).

    Returns:
      full_cls: list of (k, fw, col0, base) classes with fw full windows
      rag_cls:  list of (k, r) leftover doms per class (class order)
      rag_win:  list of (k_w, base, col0) ragged windows
      ndcp:     output columns per core
      s_elems:  slot buffer elements per core
    """
    kmax = int(counts.max()) if counts.size else 0
    n_k = np.bincount(counts, minlength=kmax + 1)
    full_cls = []
    rag_cls = []
    col = 0
    slot = 0
    for k in range(1, kmax + 1):
        if n_k[k] == 0:
            continue
        m = -(-int(n_k[k]) // NCORES)
        fw = m // 128
        r = m % 128
        if fw:
            full_cls.append((k, fw, col, slot))
            col += fw * 128
            slot += fw * 128 * k * D
        if r:
            rag_cls.append((k, r))
    # ragged windows: doms in class order (ascending k); capacity = max k in win
    rag_win = []
    R = sum(r for _, r in rag_cls)
    if R:
        ks = np.concatenate([np.full(r, k, np.int32) for k, r in rag_cls])
        RW = -(-R // 128)
        for j in range(RW):
            seg = ks[j * 128 : (j + 1) * 128]
            kw = int(seg.max())
            rag_win.append((kw, slot, col + j * 128))
            slot += 128 * kw * D
        col += RW * 128
    return full_cls, rag_cls, rag_win, col, slot


def _build_nc(full_cls, rag_win, ndcp, s_elems, nwt, nrw):
    nc = bacc.Bacc(None)
    slots_t = nc.dram_tensor("slots", [s_elems], FP16, kind="ExternalInput")
    wts_t = nc.dram_tensor("wts", [nwt * 128, D], FP16, kind="ExternalInput")
    b_t = nc.dram_tensor("b", [D, 1], FP32, kind="ExternalInput")
    if nrw:
        rt_t = nc.dram_tensor("rt", [nrw * 128, 2], FP32, kind="ExternalInput")
    out_t = nc.dram_tensor("out", [D, ndcp], FP32, kind="ExternalOutput")

    with tile.TileContext(nc) as tc:
        with (
            nc.allow_low_precision(
                reason="fp16 segment sums: |sum|<=~250, k<=~64; rel tol 2e-2"
            ),
            tc.tile_pool(name="const", bufs=1) as constp,
            tc.tile_pool(name="inp", bufs=4) as inp,
            tc.tile_pool(name="scr", bufs=2) as scrp,
            tc.tile_pool(name="mid", bufs=4) as midp,
            tc.tile_pool(name="outp", bufs=4) as outp,
            tc.tile_pool(name="psA", bufs=4, space="PSUM") as psA,
            tc.tile_pool(name="psB", bufs=4, space="PSUM") as psB,
        ):
            # scratch col sizes per fold-tree level (fp16 elems)
            SCR_SZ = [4608, 2560, 1536, 1024, 640, 384]
            ident = constp.tile([128, 128], FP16)
            make_identity(nc, ident[:])
            wt_sb = constp.tile([128, nwt * D], FP16)
            nc.sync.dma_start(
                wt_sb[:].rearrange("p (j e) -> p j e", e=D),
                wts_t[:, :].rearrange("(j p) e -> p j e", p=128),
            )
            b_sb = constp.tile([D, 1], FP32)
            nc.sync.dma_start(b_sb[:], b_t[:])
            if nrw:
                rt_sb = constp.tile([128, nrw * 2], FP32)
                nc.sync.dma_start(
                    rt_sb[:].rearrange("p (j c) -> p j c", c=2),
                    rt_t[:, :].rearrange("(j p) c -> p j c", p=128),
                )

            def window_tail(cat, out_ap, p, jwt):
                """cat: (p, 128) [sum|max] slice; out_ap: (D, p) slice."""
                catT_ps = psA.tile([128, 128], FP16, space="PSUM", tag="ps")
                nc.tensor.transpose(
                    out=catT_ps[:, :p], in_=cat, identity=ident[:p, :p]
                )
                catT = midp.tile([128, 128], FP16, tag="catT")
                nc.scalar.copy(catT[:, :p], catT_ps[:, :p])
                proj_ps = psB.tile([D, 128], FP32, space="PSUM", tag="proj")
                nc.tensor.matmul(
                    proj_ps[:, :p],
                    lhsT=wt_sb[:, jwt * D : (jwt + 1) * D],
                    rhs=catT[:, :p],
                    start=True,
                    stop=True,
                )
                nc.scalar.activation(
                    out_ap, proj_ps[:, :p],
                    mybir.ActivationFunctionType.Identity, bias=b_sb[:, :1],
                )

            # ---- full per-class windows --------------------------------
            # Slot-outer layout per group: free = (s, w, e). Pairwise fold
            # trees on DVE tensor_tensor (fp16 2x mode: ~0.52ns/out-elem vs
            # reduce's 1.06ns/in-elem). Every fold reads two contiguous
            # halves; odd levels carry the leftover slot via tensor_copy
            # (4x mode).
            def fold_tree(segs, U, op, out_ap, tag):
                """Pairwise fold over slots spread across `segs` =
                [(tile, col_off, nslots)] down to one slot -> out_ap
                [128, w, 64]. Odd leftovers stay in place (no copies);
                later levels read them from their original tile."""

                def pieces(segs_, lo, cnt):
                    out, pos = [], 0
                    for t_, off, n in segs_:
                        s0, s1 = max(lo, pos), min(lo + cnt, pos + n)
                        if s1 > s0:
                            out.append((t_, off + (s0 - pos) * U, s1 - s0))
                        pos += n
                    return out

                m = sum(n for _, _, n in segs)
                lvl = 0
                while True:
                    h, r = divmod(m, 2)
                    if h + r == 1:
                        (ta, oa, _), = pieces(segs, 0, 1)
                        (tb, ob, _), = pieces(segs, 1, 1)
                        nc.vector.tensor_tensor(
                            out=out_ap,
                            in0=ta[:, oa : oa + U].rearrange(
                                "d (w e) -> d w e", e=D
                            ),
                            in1=tb[:, ob : ob + U].rearrange(
                                "d (w e) -> d w e", e=D
                            ),
                            op=op,
                        )
                        return
                    dst = scrp.tile([128, SCR_SZ[lvl]], FP16, tag=f"{tag}{lvl}")
                    ia, ib = pieces(segs, 0, h), pieces(segs, h, h)
                    left = pieces(segs, 2 * h, r)
                    o = 0
                    while ia:
                        ta, oa, na = ia[0]
                        tb, ob, nb = ib[0]
                        cnt = min(na, nb)
                        nc.vector.tensor_tensor(
                            out=dst[:, o * U : (o + cnt) * U],
                            in0=ta[:, oa : oa + cnt * U],
                            in1=tb[:, ob : ob + cnt * U],
                            op=op,
                        )
                        o += cnt
                        ia[0] = (ta, oa + cnt * U, na - cnt)
                        ib[0] = (tb, ob + cnt * U, nb - cnt)
                        if ia[0][2] == 0:
                            ia.pop(0)
                        if ib[0][2] == 0:
                            ib.pop(0)
                    segs = [(dst, 0, h)] + left
                    m = h + r
                    lvl += 1

            for jcls, (k, fw, col0, base) in enumerate(full_cls):
                G = max(1, min(8, 128 // k))
                g = 0
                while g < fw:
                    gw = min(G, fw - g)
                    U = gw * D
                    in_t = inp.tile([128, gw * k * D], FP16, tag="in")
                    nc.sync.dma_start(
                        in_t[:],
                        slots_t[
                            base + g * 128 * k * D : base + (g + gw) * 128 * k * D
                        ].rearrange("(d f) -> d f", d=128),
                    )
                    cat_g = midp.tile([128, 8 * 128], FP16, tag="cat")
                    co = cat_g[:, : gw * 128].rearrange("d (w c) -> d w c", c=128)
                    if k == 1:
                        v = in_t[:].rearrange("d (w e) -> d w e", w=gw)
                        nc.vector.tensor_copy(co[:, :, 0:D], v)
                        nc.vector.tensor_copy(co[:, :, D:128], v)
                    else:
                        fold_tree(
                            [(in_t, 0, k)], U, mybir.AluOpType.add,
                            co[:, :, 0:D], "s",
                        )
                        fold_tree(
                            [(in_t, 0, k)], U, mybir.AluOpType.max,
                            co[:, :, D:128], "m",
                        )
                    out_sb = outp.tile([64, 8 * 128], FP32, tag="out")
                    for w in range(gw):
                        window_tail(
                            cat_g[:, w * 128 : (w + 1) * 128],
                            out_sb[:, w * 128 : (w + 1) * 128],
                            128, jcls,
                        )
                    nc.sync.dma_start(
                        out_t[:, col0 + g * 128 : col0 + (g + gw) * 128],
                        out_sb[:, : gw * 128],
                    )
                    g += gw

            # ---- ragged windows (mixed k, capacity k_w) ----------------
            juns = len(full_cls)  # unscaled weight block index
            for j, (kw, base, col0) in enumerate(rag_win):
                in_t = inp.tile([128, kw * D], FP16, tag="in")
                nc.sync.dma_start(
                    in_t[:], slots_t[base : base + 128 * kw * D].rearrange(
                        "(d f) -> d f", d=128
                    ),
                )
                cat_g = midp.tile([128, 8 * 128], FP16, tag="cat")
                sraw = midp.tile([128, D], FP16, tag="sraw")
                if kw == 1:
                    nc.vector.tensor_copy(sraw[:], in_t[:])
                    nc.vector.tensor_copy(cat_g[:, D:128], in_t[:])
                else:
                    fold_tree(
                        [(in_t, 0, kw)], D, mybir.AluOpType.add,
                        sraw[:].rearrange("d (w e) -> d w e", e=D), "s",
                    )
                    fold_tree(
                        [(in_t, 0, kw)], D, mybir.AluOpType.max,
                        cat_g[:, D:128].rearrange("d (w e) -> d w e", e=D), "m",
                    )
                # sum correction: (sraw - padcnt*x0) * recip_k (muls on ACT)
                x0 = in_t[:, 0:D]
                tmp = midp.tile([128, D], FP16, tag="tmp")
                nc.scalar.mul(tmp[:], x0, rt_sb[:, j * 2 + 1 : j * 2 + 2])
                nc.vector.tensor_tensor(
                    out=tmp[:], in0=sraw[:], in1=tmp[:], op=mybir.AluOpType.subtract
                )
                nc.scalar.mul(cat_g[:, 0:D], tmp[:], rt_sb[:, j * 2 : j * 2 + 1])
                out_sb = outp.tile([64, 8 * 128], FP32, tag="out")
                window_tail(cat_g[:, 0:128], out_sb[:, :128], 128, juns)
                nc.sync.dma_start(out_t[:, col0 : col0 + 128], out_sb[:, :128])
    nc.finalize()
    return nc


def kernel(pulse_embeddings, pulse_to_dom_idx, num_doms, proj_w, proj_b):
    global last_exec_ns
    import os

    E = np.ascontiguousarray(np.asarray(pulse_embeddings, dtype=np.float32)).astype(
        np.float16
    )
    idx = np.asarray(pulse_to_dom_idx).astype(np.int64)
    nd = int(num_doms)
    W = np.asarray(proj_w, dtype=np.float32)   # (D, 2D)
    b = np.asarray(proj_b, dtype=np.float32)   # (D,)

    counts = np.bincount(idx, minlength=nd)
    full_cls, rag_cls, rag_win, ndcp, s_elems = _plan(counts)
    nwt = len(full_cls) + 1
    nrw = len(rag_win)

    # ---- host-side dom assignment --------------------------------------
    dom_order = np.argsort(counts, kind="stable")
    cs = counts[dom_order]
    n0 = int((counts == 0).sum())
    dom_core = np.full(nd, -1, np.int32)
    dom_col = np.full(nd, -1, np.int32)

    # per-class bookkeeping (shared across cores)
    kmax = int(counts.max()) if counts.size else 0
    n_k = np.bincount(counts, minlength=kmax + 1)
    full_map = {k: (jc, fw, col0, base) for jc, (k, fw, col0, base) in enumerate(full_cls)}
    # ragged: position of each class's leftover run inside the ragged region
    rag_off = {}
    ro = 0
    for k, r in rag_cls:
        rag_off[k] = ro
        ro += r
    R = ro
    rag_col0 = rag_win[0][2] - 0 if rag_win else ndcp  # col of ragged dom 0
    if rag_win:
        rag_col0 = rag_win[0][2]

    off = n0
    # per (class-k, core): number of real doms; and split into full/ragged
    cls_meta = []  # (k, m, n_real, fw, r)
    for k in range(1, kmax + 1):
        if n_k[k] == 0:
            continue
        m = -(-int(n_k[k]) // NCORES)
        fw = m // 128
        r = m % 128
        n_real = int(n_k[k])
        doms_k = dom_order[off : off + n_real]
        off += n_real
        tot = NCORES * m
        core_of = np.arange(tot, dtype=np.int32) % NCORES
        pos_of = np.arange(tot, dtype=np.int32) // NCORES
        # column for position p: in full region if p < fw*128 else ragged
        col_full0 = full_map[k][2] if fw else 0
        p = pos_of[:n_real]
        cols = np.where(
            p < fw * 128,
            col_full0 + p,
            rag_col0 + rag_off.get(k, 0) + (p - fw * 128),
        ).astype(np.int32)
        dom_core[doms_k] = core_of[:n_real]
        dom_col[doms_k] = cols
        cls_meta.append((k, m, n_real, fw, r))

    # pulses sorted by (core, dom column)
    key = dom_core[idx].astype(np.int64) * (1 << 32) + dom_col[idx]
    perm = np.argsort(key, kind="stable")
    core_pulse_counts = np.bincount(dom_core[idx], minlength=NCORES)
    core_splits = np.concatenate([[0], np.cumsum(core_pulse_counts)])

    # ragged window lookup per ragged position
    if nrw:
        rag_kw = np.concatenate(
            [np.full(128, kw, np.int32) for kw, _, _ in rag_win]
        )[: nrw * 128]
        rag_base = np.array([bse for _, bse, _ in rag_win], np.int64)

    # ---- build per-core slot buffers ------------------------------------
    bufs = []
    for c in range(NCORES):
        buf = np.zeros(s_elems, np.float16)
        pc = perm[core_splits[c] : core_splits[c + 1]]
        p_off = 0
        # pass 1: full-window regions, in column order (= ascending k).
        # Group blocks stored slot-outer: [128 doms][slot][window][feat] so
        # the device fold-tree reads contiguous halves at every level.
        for k, m, n_real, fw, r in cls_meta:
            nreal_c = n_real // NCORES + (1 if c < n_real % NCORES else 0)
            n_full = min(nreal_c, fw * 128)
            if fw == 0 or n_full == 0:
                continue
            R_rows = pc[p_off : p_off + n_full * k].reshape(n_full, k)
            p_off += n_full * k
            base = full_map[k][3]
            Z = np.zeros((fw * 128, k, D), np.float16)
            Z[:n_full] = E[R_rows]
            G = max(1, min(8, 128 // k))
            g = 0
            while g < fw:
                gw = min(G, fw - g)
                blk = Z[g * 128 : (g + gw) * 128].reshape(gw, 128, k, D)
                off = base + g * 128 * k * D
                buf[off : off + gw * 128 * k * D] = blk.transpose(
                    1, 2, 0, 3
                ).ravel()
                g += gw
        # pass 2: ragged region, in column order (= ascending k)
        for k, m, n_real, fw, r in cls_meta:
            nreal_c = n_real // NCORES + (1 if c < n_real % NCORES else 0)
            n_full = min(nreal_c, fw * 128)
            n_rag = nreal_c - n_full
            if n_rag == 0:
                continue
            R_rows = pc[p_off : p_off + n_rag * k].reshape(n_rag, k)
            p_off += n_rag * k
            rp0 = rag_off[k]
            Arag = E[R_rows]  # (n_rag, k, D) slot-outer
            i = 0
            while i < n_rag:
                rp = rp0 + i
                j = rp // 128
                kw = int(rag_kw[rp])
                lim = min(n_rag, (j + 1) * 128 - rp0)  # same-window chunk
                chunk = Arag[i:lim]                    # (cn, k, D)
                cn = chunk.shape[0]
                blk = np.empty((cn, kw, D), np.float16)
                blk[:, :k] = chunk
                if kw > k:
                    blk[:, k:] = chunk[:, 0:1]
                bse = int(rag_base[j]) + (rp - j * 128) * D * kw
                buf[bse : bse + cn * D * kw] = blk.reshape(-1)
                i = lim
        bufs.append(buf)

    # ---- weights / tables ----------------------------------------------
    WT = np.ascontiguousarray(W.T)  # (2D, D)
    wts = np.empty((nwt * 128, D), np.float16)
    for jc, (k, fw, col0, base) in enumerate(full_cls):
        blk = WT.copy()
        blk[0:D] *= np.float32(1.0 / k)
        wts[jc * 128 : (jc + 1) * 128] = blk
    wts[len(full_cls) * 128 :] = WT  # unscaled for ragged
    b_col = b.reshape(D, 1)

    rt = None
    if nrw:
        rt = np.zeros((nrw * 128, 2), np.float32)
        rt[:, 0] = 1.0
        kd = np.zeros(nrw * 128, np.int32)
        pos = 0
        for k, r in rag_cls:
            kd[pos : pos + r] = k
            pos += r
        real = kd > 0
        rt[real, 0] = 1.0 / kd[real]
        rt[real, 1] = (rag_kw[real] - kd[real]).astype(np.float32)

    # ---- device ---------------------------------------------------------
    nc = _build_nc(full_cls, rag_win, ndcp, s_elems, nwt, nrw)
    in_maps = []
    for c in range(NCORES):
        m = {"slots": bufs[c], "wts": wts, "b": b_col}
        if nrw:
            m["rt"] = rt
        in_maps.append(m)
    trace = os.environ.get("KERNEL_TRACE", "0") == "1"
    kw_ = {}
    if trace:
        import tempfile
        kw_ = dict(trace=True, tmpdir=tempfile.mkdtemp(prefix="kernel_trace_"))
    res = run_bass_kernel_spmd(nc, in_maps, core_ids=list(range(NCORES)), **kw_)
    last_exec_ns = res.exec_time_ns

    # ---- host-side unpermute -------------------------------------------
    outs = np.stack([res.results[c]["out"] for c in range(NCORES)])  # (8, D, ndcp)
    full = np.empty((nd, D), np.float32)
    real = dom_core >= 0
    full[real] = outs[dom_core[real], :, dom_col[real]]
    if n0:
        full[~real] = b
    return full



# revision 24
# speedup vs baseline: 1.7913x; 1.0019x over previous
"""DOM pooling (segment mean+max over pulses, then linear projection) on 8 trn2 cores.

Strategy:
  Host: bucket DOMs by exact pulse count k ("classes"); deal DOMs of each class
  round-robin across the 8 cores so every core has an identical class structure
  (same per-class DOM count m_k, padded with dummy DOMs). Full 128-DOM windows
  are emitted per class; the leftover (<128) DOMs of every class are packed
  together into shared "ragged" windows (sorted by k, per-DOM slots padded to
  the window capacity by duplicating the DOM's first slot — max-neutral; the
  sum is corrected on device by subtracting padcnt*x0 and scaled by 1/k).
  Each core gets a gathered slot buffer where a DOM's k pulse embeddings are
  stored embed-major (64 x k contiguous), so device reads are sequential.

  Device (one NEFF, SPMD on 8 cores), per 128-DOM window:
    - batched contiguous DMA loads
    - DVE reduce_sum / reduce_max over the slots (contiguous inner axis)
    - PE transpose of [sum|max] concat (128x128) -> PSUM (feat-major)
    - projection matmul out^T = Wk^T.T @ concatT (mean scaling 1/k folded into
      the sum-feature rows of the per-class weights; ragged windows scale on
      DVE and use unscaled weights)
    - ACT adds bias during PSUM->SBUF copy; batched DMA writes out^T.

  Host: scatter per-core transposed outputs back to the full (num_doms, 64).
"""
import sys

import numpy as np

for _p in ("/opt/trn_rl_repo",):
    if _p not in sys.path:
        sys.path.append(_p)

from concourse import bacc
import concourse.mybir as mybir
import concourse.tile as tile
from concourse.bass_utils import run_bass_kernel_spmd
from concourse.masks import make_identity

NCORES = 8
D = 64
FP32 = mybir.dt.float32
FP16 = mybir.dt.float16

last_exec_ns = None  # set when KERNEL_TRACE=1


def _plan(counts):
    """Class/window structure shared by all cores (derived from global counts).

    Returns:
      full_cls: list of (k, fw, col0, base) classes with fw full windows
      rag_cls:  list of (k, r) leftover doms per class (class order)
      rag_win:  list of (k_w, base, col0) ragged windows
      ndcp:     output columns per core
      s_elems:  slot buffer elements per core
    """
    kmax = int(counts.max()) if counts.size else 0
    n_k = np.bincount(counts, minlength=kmax + 1)
    full_cls = []
    rag_cls = []
    col = 0
    slot = 0
    for k in range(1, kmax + 1):
        if n_k[k] == 0:
            continue
        m = -(-int(n_k[k]) // NCORES)
        fw = m // 128
        r = m % 128
        if fw:
            full_cls.append((k, fw, col, slot))
            col += fw * 128
            slot += fw * 128 * k * D
        if r:
            rag_cls.append((k, r))
    # ragged windows: doms in class order (ascending k); capacity = max k in win
    rag_win = []
    R = sum(r for _, r in rag_cls)
    if R:
        ks = np.concatenate([np.full(r, k, np.int32) for k, r in rag_cls])
        RW = -(-R // 128)
        for j in range(RW):
            seg = ks[j * 128 : (j + 1) * 128]
            kw = int(seg.max())
            rag_win.append((kw, slot, col + j * 128))
            slot += 128 * kw * D
        col += RW * 128
    return full_cls, rag_cls, rag_win, col, slot


def _build_nc(full_cls, rag_win, ndcp, s_elems, nwt, nrw):
    nc = bacc.Bacc(None)
    slots_t = nc.dram_tensor("slots", [s_elems], FP16, kind="ExternalInput")
    wts_t = nc.dram_tensor("wts", [nwt * 128, D], FP16, kind="ExternalInput")
    b_t = nc.dram_tensor("b", [D, 1], FP32, kind="ExternalInput")
    if nrw:
        rt_t = nc.dram_tensor("rt", [nrw * 128, 2], FP32, kind="ExternalInput")
    out_t = nc.dram_tensor("out", [D, ndcp], FP32, kind="ExternalOutput")

    with tile.TileContext(nc) as tc:
        with (
            nc.allow_low_precision(
                reason="fp16 segment sums: |sum|<=~250, k<=~64; rel tol 2e-2"
            ),
            tc.tile_pool(name="const", bufs=1) as constp,
            tc.tile_pool(name="inp", bufs=4) as inp,
            tc.tile_pool(name="scr", bufs=2) as scrp,
            tc.tile_pool(name="mid", bufs=6) as midp,
            tc.tile_pool(name="outp", bufs=4) as outp,
            tc.tile_pool(name="psA", bufs=4, space="PSUM") as psA,
            tc.tile_pool(name="psB", bufs=4, space="PSUM") as psB,
        ):
            # scratch col sizes per fold-tree level (fp16 elems)
            SCR_SZ = [4608, 2560, 1536, 1024, 640, 384]
            ident = constp.tile([128, 128], FP16)
            make_identity(nc, ident[:])
            wt_sb = constp.tile([128, nwt * D], FP16)
            nc.sync.dma_start(
                wt_sb[:].rearrange("p (j e) -> p j e", e=D),
                wts_t[:, :].rearrange("(j p) e -> p j e", p=128),
            )
            b_sb = constp.tile([D, 1], FP32)
            nc.sync.dma_start(b_sb[:], b_t[:])
            if nrw:
                rt_sb = constp.tile([128, nrw * 2], FP32)
                nc.sync.dma_start(
                    rt_sb[:].rearrange("p (j c) -> p j c", c=2),
                    rt_t[:, :].rearrange("(j p) c -> p j c", p=128),
                )

            def window_tail(cat, out_ap, p, jwt):
                """cat: (p, 128) [sum|max] slice; out_ap: (D, p) slice."""
                catT_ps = psA.tile([128, 128], FP16, space="PSUM", tag="ps")
                nc.tensor.transpose(
                    out=catT_ps[:, :p], in_=cat, identity=ident[:p, :p]
                )
                catT = midp.tile([128, 128], FP16, tag="catT")
                nc.scalar.copy(catT[:, :p], catT_ps[:, :p])
                proj_ps = psB.tile([D, 128], FP32, space="PSUM", tag="proj")
                nc.tensor.matmul(
                    proj_ps[:, :p],
                    lhsT=wt_sb[:, jwt * D : (jwt + 1) * D],
                    rhs=catT[:, :p],
                    start=True,
                    stop=True,
                )
                nc.scalar.activation(
                    out_ap, proj_ps[:, :p],
                    mybir.ActivationFunctionType.Identity, bias=b_sb[:, :1],
                )

            # ---- full per-class windows --------------------------------
            # Slot-outer layout per group: free = (s, w, e). Pairwise fold
            # trees on DVE tensor_tensor (fp16 2x mode: ~0.52ns/out-elem vs
            # reduce's 1.06ns/in-elem). Every fold reads two contiguous
            # halves; odd levels carry the leftover slot via tensor_copy
            # (4x mode).
            def fold_tree(segs, U, op, out_ap, tag):
                """Pairwise fold over slots spread across `segs` =
                [(tile, col_off, nslots)] down to one slot -> out_ap
                [128, w, 64]. Odd leftovers stay in place (no copies);
                later levels read them from their original tile."""

                def pieces(segs_, lo, cnt):
                    out, pos = [], 0
                    for t_, off, n in segs_:
                        s0, s1 = max(lo, pos), min(lo + cnt, pos + n)
                        if s1 > s0:
                            out.append((t_, off + (s0 - pos) * U, s1 - s0))
                        pos += n
                    return out

                m = sum(n for _, _, n in segs)
                lvl = 0
                while True:
                    h, r = divmod(m, 2)
                    if h + r == 1:
                        (ta, oa, _), = pieces(segs, 0, 1)
                        (tb, ob, _), = pieces(segs, 1, 1)
                        nc.vector.tensor_tensor(
                            out=out_ap,
                            in0=ta[:, oa : oa + U].rearrange(
                                "d (w e) -> d w e", e=D
                            ),
                            in1=tb[:, ob : ob + U].rearrange(
                                "d (w e) -> d w e", e=D
                            ),
                            op=op,
                        )
                        return
                    dst = scrp.tile([128, SCR_SZ[lvl]], FP16, tag=f"{tag}{lvl}")
                    ia, ib = pieces(segs, 0, h), pieces(segs, h, h)
                    left = pieces(segs, 2 * h, r)
                    o = 0
                    while ia:
                        ta, oa, na = ia[0]
                        tb, ob, nb = ib[0]
                        cnt = min(na, nb)
                        nc.vector.tensor_tensor(
                            out=dst[:, o * U : (o + cnt) * U],
                            in0=ta[:, oa : oa + cnt * U],
                            in1=tb[:, ob : ob + cnt * U],
                            op=op,
                        )
                        o += cnt
                        ia[0] = (ta, oa + cnt * U, na - cnt)
                        ib[0] = (tb, ob + cnt * U, nb - cnt)
                        if ia[0][2] == 0:
                            ia.pop(0)
                        if ib[0][2] == 0:
                            ib.pop(0)
                    segs = [(dst, 0, h)] + left
                    m = h + r
                    lvl += 1

            def emit_full_group(jcls, k, col0, base, g, gw):
                U = gw * D
                in_t = inp.tile([128, gw * k * D], FP16, tag="in")
                nc.sync.dma_start(
                    in_t[:],
                    slots_t[
                        base + g * 128 * k * D : base + (g + gw) * 128 * k * D
                    ].rearrange("(d f) -> d f", d=128),
                )
                cat_g = midp.tile([128, 8 * 128], FP16, tag="cat")
                co = cat_g[:, : gw * 128].rearrange("d (w c) -> d w c", c=128)
                if k == 1:
                    v = in_t[:].rearrange("d (w e) -> d w e", w=gw)
                    nc.vector.tensor_copy(co[:, :, 0:D], v)
                    nc.vector.tensor_copy(co[:, :, D:128], v)
                else:
                    fold_tree(
                        [(in_t, 0, k)], U, mybir.AluOpType.add,
                        co[:, :, 0:D], "s",
                    )
                    fold_tree(
                        [(in_t, 0, k)], U, mybir.AluOpType.max,
                        co[:, :, D:128], "m",
                    )
                out_sb = outp.tile([64, 8 * 128], FP32, tag="out")
                for w in range(gw):
                    window_tail(
                        cat_g[:, w * 128 : (w + 1) * 128],
                        out_sb[:, w * 128 : (w + 1) * 128],
                        128, jcls,
                    )
                nc.sync.dma_start(
                    out_t[:, col0 + g * 128 : col0 + (g + gw) * 128],
                    out_sb[:, : gw * 128],
                )

            juns = len(full_cls)  # unscaled weight block index

            def emit_ragged(j, kw, base, col0):
                in_t = inp.tile([128, kw * D], FP16, tag="in")
                nc.sync.dma_start(
                    in_t[:], slots_t[base : base + 128 * kw * D].rearrange(
                        "(d f) -> d f", d=128
                    ),
                )
                cat_g = midp.tile([128, 8 * 128], FP16, tag="cat")
                sraw = midp.tile([128, D], FP16, tag="sraw")
                if kw == 1:
                    nc.vector.tensor_copy(sraw[:], in_t[:])
                    nc.vector.tensor_copy(cat_g[:, D:128], in_t[:])
                else:
                    fold_tree(
                        [(in_t, 0, kw)], D, mybir.AluOpType.add,
                        sraw[:].rearrange("d (w e) -> d w e", e=D), "s",
                    )
                    fold_tree(
                        [(in_t, 0, kw)], D, mybir.AluOpType.max,
                        cat_g[:, D:128].rearrange("d (w e) -> d w e", e=D), "m",
                    )
                # sum correction: (sraw - padcnt*x0) * recip_k (muls on ACT)
                x0 = in_t[:, 0:D]
                tmp = midp.tile([128, D], FP16, tag="tmp")
                nc.scalar.mul(tmp[:], x0, rt_sb[:, j * 2 + 1 : j * 2 + 2])
                nc.vector.tensor_tensor(
                    out=tmp[:], in0=sraw[:], in1=tmp[:], op=mybir.AluOpType.subtract
                )
                nc.scalar.mul(cat_g[:, 0:D], tmp[:], rt_sb[:, j * 2 : j * 2 + 1])
                out_sb = outp.tile([64, 8 * 128], FP32, tag="out")
                window_tail(cat_g[:, 0:128], out_sb[:, :128], 128, juns)
                nc.sync.dma_start(out_t[:, col0 : col0 + 128], out_sb[:, :128])

            # ---- emission: ragged windows interleaved among full groups
            # (clustering them at the end serializes their cross-engine
            # correction chains; spreading hides them under tree work)
            full_jobs = []
            for jcls, (k, fw, col0, base) in enumerate(full_cls):
                G = max(1, min(8, 128 // k))
                g = 0
                while g < fw:
                    gw = min(G, fw - g)
                    full_jobs.append((jcls, k, col0, base, g, gw))
                    g += gw
            rag_jobs = list(enumerate(rag_win))
            stride = max(1, len(full_jobs) // (len(rag_jobs) + 1))
            ri = 0
            for i, job in enumerate(full_jobs):
                emit_full_group(*job)
                if i % stride == stride - 1 and ri < len(rag_jobs):
                    j, (kw, base, col0) = rag_jobs[ri]
                    emit_ragged(j, kw, base, col0)
                    ri += 1
            while ri < len(rag_jobs):
                j, (kw, base, col0) = rag_jobs[ri]
                emit_ragged(j, kw, base, col0)
                ri += 1
    nc.finalize()
    return nc


def kernel(pulse_embeddings, pulse_to_dom_idx, num_doms, proj_w, proj_b):
    global last_exec_ns
    import os

    E = np.ascontiguousarray(np.asarray(pulse_embeddings, dtype=np.float32)).astype(
        np.float16
    )
    idx = np.asarray(pulse_to_dom_idx).astype(np.int64)
    nd = int(num_doms)
    W = np.asarray(proj_w, dtype=np.float32)   # (D, 2D)
    b = np.asarray(proj_b, dtype=np.float32)   # (D,)

    counts = np.bincount(idx, minlength=nd)
    full_cls, rag_cls, rag_win, ndcp, s_elems = _plan(counts)
    nwt = len(full_cls) + 1
    nrw = len(rag_win)

    # ---- host-side dom assignment --------------------------------------
    dom_order = np.argsort(counts, kind="stable")
    cs = counts[dom_order]
    n0 = int((counts == 0).sum())
    dom_core = np.full(nd, -1, np.int32)
    dom_col = np.full(nd, -1, np.int32)

    # per-class bookkeeping (shared across cores)
    kmax = int(counts.max()) if counts.size else 0
    n_k = np.bincount(counts, minlength=kmax + 1)
    full_map = {k: (jc, fw, col0, base) for jc, (k, fw, col0, base) in enumerate(full_cls)}
    # ragged: position of each class's leftover run inside the ragged region
    rag_off = {}
    ro = 0
    for k, r in rag_cls:
        rag_off[k] = ro
        ro += r
    R = ro
    rag_col0 = rag_win[0][2] - 0 if rag_win else ndcp  # col of ragged dom 0
    if rag_win:
        rag_col0 = rag_win[0][2]

    off = n0
    # per (class-k, core): number of real doms; and split into full/ragged
    cls_meta = []  # (k, m, n_real, fw, r)
    for k in range(1, kmax + 1):
        if n_k[k] == 0:
            continue
        m = -(-int(n_k[k]) // NCORES)
        fw = m // 128
        r = m % 128
        n_real = int(n_k[k])
        doms_k = dom_order[off : off + n_real]
        off += n_real
        tot = NCORES * m
        core_of = np.arange(tot, dtype=np.int32) % NCORES
        pos_of = np.arange(tot, dtype=np.int32) // NCORES
        # column for position p: in full region if p < fw*128 else ragged
        col_full0 = full_map[k][2] if fw else 0
        p = pos_of[:n_real]
        cols = np.where(
            p < fw * 128,
            col_full0 + p,
            rag_col0 + rag_off.get(k, 0) + (p - fw * 128),
        ).astype(np.int32)
        dom_core[doms_k] = core_of[:n_real]
        dom_col[doms_k] = cols
        cls_meta.append((k, m, n_real, fw, r))

    # pulses sorted by (core, dom column)
    key = dom_core[idx].astype(np.int64) * (1 << 32) + dom_col[idx]
    perm = np.argsort(key, kind="stable")
    core_pulse_counts = np.bincount(dom_core[idx], minlength=NCORES)
    core_splits = np.concatenate([[0], np.cumsum(core_pulse_counts)])

    # ragged window lookup per ragged position
    if nrw:
        rag_kw = np.concatenate(
            [np.full(128, kw, np.int32) for kw, _, _ in rag_win]
        )[: nrw * 128]
        rag_base = np.array([bse for _, bse, _ in rag_win], np.int64)

    # ---- build per-core slot buffers ------------------------------------
    bufs = []
    for c in range(NCORES):
        buf = np.zeros(s_elems, np.float16)
        pc = perm[core_splits[c] : core_splits[c + 1]]
        p_off = 0
        # pass 1: full-window regions, in column order (= ascending k).
        # Group blocks stored slot-outer: [128 doms][slot][window][feat] so
        # the device fold-tree reads contiguous halves at every level.
        for k, m, n_real, fw, r in cls_meta:
            nreal_c = n_real // NCORES + (1 if c < n_real % NCORES else 0)
            n_full = min(nreal_c, fw * 128)
            if fw == 0 or n_full == 0:
                continue
            R_rows = pc[p_off : p_off + n_full * k].reshape(n_full, k)
            p_off += n_full * k
            base = full_map[k][3]
            Z = np.zeros((fw * 128, k, D), np.float16)
            Z[:n_full] = E[R_rows]
            G = max(1, min(8, 128 // k))
            g = 0
            while g < fw:
                gw = min(G, fw - g)
                blk = Z[g * 128 : (g + gw) * 128].reshape(gw, 128, k, D)
                off = base + g * 128 * k * D
                buf[off : off + gw * 128 * k * D] = blk.transpose(
                    1, 2, 0, 3
                ).ravel()
                g += gw
        # pass 2: ragged region, in column order (= ascending k)
        for k, m, n_real, fw, r in cls_meta:
            nreal_c = n_real // NCORES + (1 if c < n_real % NCORES else 0)
            n_full = min(nreal_c, fw * 128)
            n_rag = nreal_c - n_full
            if n_rag == 0:
                continue
            R_rows = pc[p_off : p_off + n_rag * k].reshape(n_rag, k)
            p_off += n_rag * k
            rp0 = rag_off[k]
            Arag = E[R_rows]  # (n_rag, k, D) slot-outer
            i = 0
            while i < n_rag:
                rp = rp0 + i
                j = rp // 128
                kw = int(rag_kw[rp])
                lim = min(n_rag, (j + 1) * 128 - rp0)  # same-window chunk
                chunk = Arag[i:lim]                    # (cn, k, D)
                cn = chunk.shape[0]
                blk = np.empty((cn, kw, D), np.float16)
                blk[:, :k] = chunk
                if kw > k:
                    blk[:, k:] = chunk[:, 0:1]
                bse = int(rag_base[j]) + (rp - j * 128) * D * kw
                buf[bse : bse + cn * D * kw] = blk.reshape(-1)
                i = lim
        bufs.append(buf)

    # ---- weights / tables ----------------------------------------------
    WT = np.ascontiguousarray(W.T)  # (2D, D)
    wts = np.empty((nwt * 128, D), np.float16)
    for jc, (k, fw, col0, base) in enumerate(full_cls):
        blk = WT.copy()
        blk[0:D] *= np.float32(1.0 / k)
        wts[jc * 128 : (jc + 1) * 128] = blk
    wts[len(full_cls) * 128 :] = WT  # unscaled for ragged
    b_col = b.reshape(D, 1)

    rt = None
    if nrw:
        rt = np.zeros((nrw * 128, 2), np.float32)
        rt[:, 0] = 1.0
        kd = np.zeros(nrw * 128, np.int32)
        pos = 0
        for k, r in rag_cls:
            kd[pos : pos + r] = k
            pos += r
        real = kd > 0
        rt[real, 0] = 1.0 / kd[real]
        rt[real, 1] = (rag_kw[real] - kd[real]).astype(np.float32)

    # ---- device ---------------------------------------------------------
    nc = _build_nc(full_cls, rag_win, ndcp, s_elems, nwt, nrw)
    in_maps = []
    for c in range(NCORES):
        m = {"slots": bufs[c], "wts": wts, "b": b_col}
        if nrw:
            m["rt"] = rt
        in_maps.append(m)
    trace = os.environ.get("KERNEL_TRACE", "0") == "1"
    kw_ = {}
    if trace:
        import tempfile
        kw_ = dict(trace=True, tmpdir=tempfile.mkdtemp(prefix="kernel_trace_"))
    res = run_bass_kernel_spmd(nc, in_maps, core_ids=list(range(NCORES)), **kw_)
    last_exec_ns = res.exec_time_ns

    # ---- host-side unpermute -------------------------------------------
    outs = np.stack([res.results[c]["out"] for c in range(NCORES)])  # (8, D, ndcp)
    full = np.empty((nd, D), np.float32)
    real = dom_core >= 0
    full[real] = outs[dom_core[real], :, dom_col[real]]
    if n0:
        full[~real] = b
    return full

